# revision 12
# baseline (speedup 1.0000x reference)
"""Axial (frame-local) multi-head attention for Trainium2, 8-core SPMD.

Problem: x:[2,8192,512] -> qkv proj -> per-(batch,head,frame) attention over
n=1024 tokens -> out proj. B=2, f=8 frames, h=8 heads, d=64.

Sharding: the 16 (batch, frame) pairs are embarrassingly parallel; each of
the 8 cores handles 2 pairs end-to-end (weights replicated). Host
pre-transposes x (ch-major) and pre-casts everything to bf16 so every
on-chip matmul is a 1-cycle/column bf16 op.

v2 design (vs 260us baseline): the kernel is ACT(exp)-and-PE-bound, so
- all projections in bf16 (f32r gone), all DMA halved
- stage B is it-outer: one [128,2048] PSUM tile holds 4 sims (2 heads x
  2 j-blocks); ONE 2048-col exp per 4 sims -> ACT ~121us with minimal
  per-instruction access bubbles
- PSUM budget = psAB(4 banks) + po_A + po_B + 2-bank filler pool, so the
  qkv/out projections of the *other* pair interleave INSIDE stage B's
  ACT-bound envelope instead of serializing before/after it
- softmax denominators: ones-column rides the AV matmul (row 64 of po);
  rows packed to [8,512], reciprocal_approx_fast (5x cheaper than DVE
  reciprocal), bf16 DRAM-bounce broadcast, one [128,512] DVE multiply per
  (hp, it)
"""
import json
import numpy as np
from contextlib import ExitStack

import concourse.bass as bass
import concourse.tile as tile
import concourse.mybir as mybir
from concourse.bass_utils import run_bass_kernel_spmd

F32 = mybir.dt.float32
BF16 = mybir.dt.bfloat16
AF = mybir.ActivationFunctionType

B, NTOT, DIM = 2, 8192, 512
H, D, F = 8, 64, 8
N = NTOT // F            # 1024 tokens per frame
SCALE = D ** -0.5
NP = 2                   # (batch, frame) pairs per core
TOK = NP * N             # 2048 tokens per core

DT = BF16


def _legalize_waits(bir: bytes) -> bytes:
    """TRN2 instructions carry a single HW wait slot and this walrus build
    refuses to split multi-wait instructions; hoist extra waits onto NoOps
    inserted just before, on the same engine stream."""
    j = json.loads(bir)
    ctr = 0
    for fn in j["functions"]:
        for blk in fn["blocks"]:
            out = []
            for inst in blk["instructions"]:
                si = inst.get("sync_info")
                if si:
                    waits = si.get("on_wait") or []
                    if len(waits) > 1:
                        for w in waits[:-1]:
                            ctr += 1
                            nop = {
                                "engine": inst["engine"],
                                "ins": [], "outs": [],
                                "name": f"I-waitfix-{ctr}",
                                "opcode": "NoOp",
                                "sync_info": {"on_update": [], "on_wait": [w]},
                            }
                            if "debug" in inst:
                                nop["debug"] = inst["debug"]
                            out.append(nop)
                        si["on_wait"] = waits[-1:]
                out.append(inst)
            blk["instructions"] = out
    return json.dumps(j).encode()


def build(with_bias=True):
    nc = bass.Bass(trn_type="TRN2")
    xt = nc.dram_tensor("xt", [DIM, TOK], BF16, kind="ExternalInput")
    wqk = nc.dram_tensor("wqk", [DIM, 1024], BF16, kind="ExternalInput")
    wv = nc.dram_tensor("wv", [DIM, 512], BF16, kind="ExternalInput")
    wout = nc.dram_tensor("wout", [DIM, 512], BF16, kind="ExternalInput")
    bout = nc.dram_tensor("bout", [1, 512], F32, kind="ExternalInput")
    y = nc.dram_tensor("y", [TOK, DIM], F32, kind="ExternalOutput")
    # den bounce scratch, one per (pair, it); full-tile stores + row-broadcast
    # loads (slice-stores raced the loads)
    dscr = [nc.dram_tensor(f"dscr{t}", [128, 1024], BF16) for t in range(4)]

    with tile.TileContext(nc) as tc, ExitStack() as ctx:
        const = ctx.enter_context(tc.tile_pool(name="const", bufs=1))
        qk_pool = ctx.enter_context(tc.tile_pool(name="qk", bufs=1))
        vv_pool = ctx.enter_context(tc.tile_pool(name="vv", bufs=1))
        et_pool = ctx.enter_context(tc.tile_pool(name="et", bufs=3))
        otn_pool = ctx.enter_context(tc.tile_pool(name="otn", bufs=1))
        den_pool = ctx.enter_context(tc.tile_pool(name="den", bufs=1))
        rb_pool = ctx.enter_context(tc.tile_pool(name="rb", bufs=4))
        y_pool = ctx.enter_context(tc.tile_pool(name="yo", bufs=3))
        ps = ctx.enter_context(tc.tile_pool(name="ps", bufs=1, space="PSUM"))

        # ---- weights / x / bias ----
        wqk_sb = [const.tile([128, 1024], BF16, tag=f"wqk{k}", name=f"wqk{k}")
                  for k in range(4)]
        wv_sb = [const.tile([128, 512], BF16, tag=f"wv{k}", name=f"wv{k}")
                 for k in range(4)]
        wout_sb = [const.tile([128, 512], BF16, tag=f"wout{k}", name=f"wout{k}")
                   for k in range(4)]
        xt_sb = [const.tile([128, TOK], BF16, tag=f"xt{k}", name=f"xt{k}")
                 for k in range(4)]
        bias_sb = const.tile([128, 512], F32, tag="bias", name="bias")
        for k in range(4):
            nc.sync.dma_start(wqk_sb[k][:], wqk.ap()[k * 128:(k + 1) * 128, :])
        for k in range(4):
            nc.sync.dma_start(xt_sb[k][:, 0:N], xt.ap()[k * 128:(k + 1) * 128, 0:N])
        for k in range(4):
            nc.sync.dma_start(wv_sb[k][:], wv.ap()[k * 128:(k + 1) * 128, :])
        for k in range(4):
            nc.sync.dma_start(xt_sb[k][:, N:TOK], xt.ap()[k * 128:(k + 1) * 128, N:TOK])
        for k in range(4):
            nc.sync.dma_start(wout_sb[k][:], wout.ap()[k * 128:(k + 1) * 128, :])
        if with_bias:
            nc.sync.dma_start(bias_sb[:], bout.ap().broadcast_to([128, 512]))

        ones8_f = const.tile([128, 8], F32, tag="ones8f", name="ones8f")
        nc.gpsimd.memset(ones8_f[:], 1.0)
        ones8 = const.tile([128, 8], DT, tag="ones8", name="ones8")
        nc.vector.tensor_copy(ones8[:], ones8_f[:])

        # per-pair on-chip tensors
        qkT = {p: [qk_pool.tile([128, N], DT, tag=f"qkT{p}_{c}", name=f"qkT{p}_{c}")
                   for c in range(8)] for p in range(NP)}
        vv = {p: [vv_pool.tile([128, 8 * 65], DT, tag=f"vv{p}_{t}", name=f"vv{p}_{t}")
                  for t in range(8)] for p in range(NP)}
        otn = {p: [otn_pool.tile([128, N], DT, tag=f"otn{p}_{h}", name=f"otn{p}_{h}")
                   for h in range(4)] for p in range(NP)}
        otn_u = {p: [otn_pool.tile([128, N], DT, tag=f"otu{p}_{h}", name=f"otu{p}_{h}")
                     for h in range(4)] for p in range(NP)}
        # den rows live at 32-aligned partitions (engine APs require it):
        # row 32*hp, cols 0:512 head-A den, cols 512:1024 head-B den.
        den = {(p, it): den_pool.tile([128, 1024], F32, tag=f"den{p}_{it}",
                                      name=f"den{p}_{it}")
               for p in range(NP) for it in range(2)}
        rcpf = {(p, it): den_pool.tile([128, 1024], F32, tag=f"rcf{p}_{it}",
                                       name=f"rcf{p}_{it}")
                for p in range(NP) for it in range(2)}
        rcpb = {(p, it): den_pool.tile([128, 1024], BF16, tag=f"rcb{p}_{it}",
                                       name=f"rcb{p}_{it}")
                for p in range(NP) for it in range(2)}

        # ---------- filler chains (projections + out-proj), pumped into B ----
        def chain_qk(p, cht, nt):
            """qkT[p][cht][:, nt*512:(nt+1)*512] ch-major projection chain."""
            def emit(on_act=False):
                t0 = p * N
                pa = ps.tile([128, 512], F32, tag="pa", name="pa", bufs=2)
                for kt in range(4):
                    nc.tensor.matmul(
                        pa[:],
                        wqk_sb[kt][:, cht * 128:(cht + 1) * 128],
                        xt_sb[kt][:, t0 + nt * 512:t0 + (nt + 1) * 512],
                        start=(kt == 0), stop=(kt == 3))
                dst = qkT[p][cht][:, nt * 512:(nt + 1) * 512]
                if on_act:
                    nc.scalar.copy(dst, pa[:])
                else:
                    nc.vector.tensor_copy(dst, pa[:])
            return emit

        def chain_v(p, tt):
            """vv[p][tt]: tok-major v, heads packed at stride 65 + ones col."""
            def emit(on_act=False):
                t0 = p * N
                pv = ps.tile([128, 512], F32, tag="pa", name="pa", bufs=2)
                for kt in range(4):
                    nc.tensor.matmul(
                        pv[:],
                        xt_sb[kt][:, t0 + tt * 128:t0 + (tt + 1) * 128],
                        wv_sb[kt][:],
                        start=(kt == 0), stop=(kt == 3))
                dst = vv[p][tt][:].rearrange("p (h c) -> p h c", c=65)[:, :, 0:64]
                src = pv[:].rearrange("p (h c) -> p h c", c=64)
                nc.vector.tensor_copy(dst, src)
                ones_dst = vv[p][tt][:].rearrange("p (h c) -> p h c", c=65)[:, :, 64:65]
                ones_src = ones8[:].rearrange("p (h c) -> p h c", c=1)
                nc.vector.tensor_copy(ones_dst, ones_src)
            return emit

        def chain_c(p, it, tt):
            """out projection for token chunk (p, it, tt) + bias + store."""
            def emit(on_act=False):
                py = ps.tile([128, 512], F32, tag="pa", name="pa", bufs=2)
                for kt in range(4):
                    nc.tensor.matmul(
                        py[:],
                        otn[p][kt][:, it * 512 + tt * 128:it * 512 + (tt + 1) * 128],
                        wout_sb[kt][:],
                        start=(kt == 0), stop=(kt == 3))
                ysb = y_pool.tile([128, 512], F32, tag="ysb", name="ysb")
                if with_bias:
                    nc.vector.tensor_add(ysb[:], py[:], bias_sb[:])
                else:
                    nc.vector.tensor_copy(ysb[:], py[:])
                r0 = p * N + it * 512 + tt * 128
                nc.sync.dma_start(y.ap()[r0:r0 + 128, :], ysb[:])
            return emit

        # ---------- stage B ----------
        def emit_B(p, it, hp, pump):
            hA, hB = 2 * hp, 2 * hp + 1
            qt, kt_t = qkT[p][hp], qkT[p][4 + hp]
            po_A = ps.tile([128, 512], F32, tag="poA", name="poA", bufs=1)
            po_B = ps.tile([128, 512], F32, tag="poB", name="poB", bufs=1)
            for jp in range(4):
                psAB = ps.tile([128, 2048], F32, tag="psAB", name="psAB", bufs=1)
                for u in range(2):
                    jt = 2 * jp + u
                    nc.tensor.matmul(
                        psAB[:, (2 * u) * 512:(2 * u + 1) * 512],
                        kt_t[0:64, jt * 128:(jt + 1) * 128],
                        qt[0:64, it * 512:(it + 1) * 512],
                        start=True, stop=True, tile_position=(0, 0))
                    nc.tensor.matmul(
                        psAB[:, (2 * u + 1) * 512:(2 * u + 2) * 512],
                        kt_t[64:128, jt * 128:(jt + 1) * 128],
                        qt[64:128, it * 512:(it + 1) * 512],
                        start=True, stop=True, tile_position=(64, 0))
                et = et_pool.tile([128, 2048], DT, tag="et", name="et")
                nc.scalar.activation(et[:], psAB[:], AF.Exp)
                for u in range(2):
                    jt = 2 * jp + u
                    nc.tensor.matmul(
                        po_A[0:65, :],
                        vv[p][jt][:, hA * 65:hA * 65 + 65],
                        et[:, (2 * u) * 512:(2 * u + 1) * 512],
                        start=(jp == 0 and u == 0), stop=(jp == 3 and u == 1))
                    nc.tensor.matmul(
                        po_B[0:65, :],
                        vv[p][jt][:, hB * 65:hB * 65 + 65],
                        et[:, (2 * u + 1) * 512:(2 * u + 2) * 512],
                        start=(jp == 0 and u == 0), stop=(jp == 3 and u == 1))
                pump()
            # denominators + numerator eviction (frees po banks)
            nc.vector.tensor_copy(den[(p, it)][32 * hp:32 * hp + 1, 0:512],
                                  po_A[64:65, :])
            nc.vector.tensor_copy(den[(p, it)][32 * hp:32 * hp + 1, 512:1024],
                                  po_B[64:65, :])
            nc.vector.tensor_copy(otn_u[p][hp][0:64, it * 512:(it + 1) * 512],
                                  po_A[0:64, :])
            nc.vector.tensor_copy(otn_u[p][hp][64:128, it * 512:(it + 1) * 512],
                                  po_B[0:64, :])

        def emit_norm(p, it):
            """recip + DRAM-bounce broadcast + per-hp normalize for (p, it).

            Only the 32-aligned rows of den/rcp tiles hold data; the recip of
            the untouched lanes is garbage that is never read."""
            sc = dscr[p * 2 + it]
            nc.vector.reciprocal(rcpf[(p, it)][:], den[(p, it)][:])
            nc.vector.tensor_copy(rcpb[(p, it)][:], rcpf[(p, it)][:])
            nc.sync.dma_start(sc.ap(), rcpb[(p, it)][:])
            for hp in range(4):
                rb = rb_pool.tile([128, 512], BF16, tag="rb", name="rb")
                nc.sync.dma_start(
                    rb[0:64, :],
                    sc.ap()[32 * hp:32 * hp + 1, 0:512].broadcast_to([64, 512]))
                nc.sync.dma_start(
                    rb[64:128, :],
                    sc.ap()[32 * hp:32 * hp + 1, 512:1024].broadcast_to([64, 512]))
                nc.vector.tensor_mul(otn[p][hp][:, it * 512:(it + 1) * 512],
                                     otn_u[p][hp][:, it * 512:(it + 1) * 512],
                                     rb[:])

        # ---------- schedule ----------
        # A(p) chain order, with the index every B(p, it, hp) requires
        # emitted beforehand (emission order IS dependency order for Tile):
        #   0-10 : v0..v7, k(hp0) both halves, q(hp0) it0  -> B(p,0,0)
        #   11-13: k(hp1), q(hp1) it0                      -> B(p,0,1)  etc.
        #   20+hp: q(hp) it1                               -> B(p,1,hp)
        def a_chains(p):
            out = [(chain_v(p, t), False) for t in range(8)]
            out += [(chain_qk(p, 4, 0), True), (chain_qk(p, 4, 1), True),
                    (chain_qk(p, 0, 0), True)]
            for hp in range(1, 4):
                out += [(chain_qk(p, 4 + hp, 0), False),
                        (chain_qk(p, 4 + hp, 1), False),
                        (chain_qk(p, hp, 0), False)]
            out += [(chain_qk(p, hp, 1), False) for hp in range(4)]
            return out

        Aq = {0: a_chains(0), 1: a_chains(1)}
        Ai = {0: 0, 1: 0}
        Cq = []

        def drainA(p, n, act_assist=False):
            while Ai[p] < n:
                fn, is_qk = Aq[p][Ai[p]]
                fn(on_act=(act_assist and is_qk))
                Ai[p] += 1

        def pump():
            if Ai[0] < len(Aq[0]):
                Aq[0][Ai[0]][0]()
                Ai[0] += 1
            elif Ai[1] < len(Aq[1]):
                Aq[1][Ai[1]][0]()
                Ai[1] += 1
            elif Cq:
                Cq.pop(0)()

        def req(it, hp):
            return (11 + 3 * hp) if it == 0 else (21 + hp)

        def run_B(p, it):
            for hp in range(4):
                drainA(p, req(it, hp), act_assist=(p == 0 and it == 0))
                emit_B(p, it, hp, pump)
            emit_norm(p, it)

        run_B(0, 0)
        run_B(0, 1)
        Cq += [chain_c(0, 0, tt) for tt in range(4)]
        run_B(1, 0)
        Cq += [chain_c(0, 1, tt) for tt in range(4)]
        Cq += [chain_c(1, 0, tt) for tt in range(4)]
        run_B(1, 1)
        while Ai[0] < len(Aq[0]) or Ai[1] < len(Aq[1]) or Cq:
            pump()
        for tt in range(4):
            chain_c(1, 1, tt)()

    _orig = nc.to_json_bytes
    nc.to_json_bytes = lambda: _legalize_waits(_orig())
    return nc


_NC_CACHE = []
_last_in_maps = None


def kernel(**inputs) -> np.ndarray:
    import ml_dtypes
    BF = ml_dtypes.bfloat16

    x = np.ascontiguousarray(np.asarray(inputs["x"], dtype=np.float32))
    W_qkv = np.asarray(inputs["W_qkv"], dtype=np.float32)
    W_out = np.ascontiguousarray(np.asarray(inputs["W_out"], dtype=np.float32))
    b_out = np.ascontiguousarray(np.asarray(inputs["b_out"], dtype=np.float32))
    f = int(np.asarray(inputs["f"]))
    assert f == F and x.shape == (B, NTOT, DIM)

    Wqk = np.ascontiguousarray(np.concatenate(
        [W_qkv[:, :512] * SCALE, W_qkv[:, 512:1024]], axis=1).astype(BF))
    Wv = np.ascontiguousarray(W_qkv[:, 1024:1536].astype(BF))
    Wo = np.ascontiguousarray(W_out.astype(BF))
    bo = np.ascontiguousarray(b_out.reshape(1, 512))

    with_bias = bool(np.any(b_out))
    key = with_bias
    if not _NC_CACHE or _NC_CACHE[0][0] != key:
        _NC_CACHE.clear()
        _NC_CACHE.append((key, build(with_bias)))
    nc = _NC_CACHE[0][1]

    in_maps = []
    for core in range(8):
        pairs = (2 * core, 2 * core + 1)
        xT = np.concatenate(
            [x[p // F, (p % F) * N:(p % F + 1) * N, :].T for p in pairs],
            axis=1).astype(BF)
        in_maps.append({
            "xt": np.ascontiguousarray(xT),
            "wqk": Wqk, "wv": Wv, "wout": Wo, "bout": bo,
        })

    global _last_in_maps
    _last_in_maps = in_maps
    try:
        res = run_bass_kernel_spmd(nc, in_maps, list(range(8)))
    except Exception:
        # transient NRT_EXEC_UNIT_UNRECOVERABLE occasionally hits the first
        # submission after a fresh compile; one retry has always cleared it
        import time
        time.sleep(10)
        res = run_bass_kernel_spmd(nc, in_maps, list(range(8)))

    out = np.zeros((B, NTOT, DIM), dtype=np.float32)
    for core in range(8):
        yc = res.results[core]["y"]
        for pi, p in enumerate((2 * core, 2 * core + 1)):
            out[p // F, (p % F) * N:(p % F + 1) * N, :] = yc[pi * N:(pi + 1) * N]
    return out


# revision 17
# speedup vs baseline: 1.1030x; 1.1030x over previous
"""Axial (frame-local) multi-head attention for Trainium2, 8-core SPMD.

Problem: x:[2,8192,512] -> qkv proj -> per-(batch,head,frame) attention over
n=1024 tokens -> out proj. B=2, f=8 frames, h=8 heads, d=64.

Sharding: the 16 (batch, frame) pairs are embarrassingly parallel; each of
the 8 cores handles 2 pairs end-to-end (weights replicated). Host
pre-transposes x (ch-major) and pre-casts everything to bf16 so every
on-chip matmul is a 1-cycle/column bf16 op.

v2 design (vs 260us baseline): the kernel is ACT(exp)-and-PE-bound, so
- all projections in bf16 (f32r gone), all DMA halved
- stage B is it-outer: one [128,2048] PSUM tile holds 4 sims (2 heads x
  2 j-blocks); ONE 2048-col exp per 4 sims -> ACT ~121us with minimal
  per-instruction access bubbles
- PSUM budget = psAB(4 banks) + po_A + po_B + 2-bank filler pool, so the
  qkv/out projections of the *other* pair interleave INSIDE stage B's
  ACT-bound envelope instead of serializing before/after it
- softmax denominators: ones-column rides the AV matmul (row 64 of po);
  rows packed to [8,512], reciprocal_approx_fast (5x cheaper than DVE
  reciprocal), bf16 DRAM-bounce broadcast, one [128,512] DVE multiply per
  (hp, it)
"""
import json
import numpy as np
from contextlib import ExitStack

import concourse.bass as bass
import concourse.tile as tile
import concourse.mybir as mybir
from concourse.bass_utils import run_bass_kernel_spmd

F32 = mybir.dt.float32
BF16 = mybir.dt.bfloat16
AF = mybir.ActivationFunctionType

B, NTOT, DIM = 2, 8192, 512
H, D, F = 8, 64, 8
N = NTOT // F            # 1024 tokens per frame
SCALE = D ** -0.5
NP = 2                   # (batch, frame) pairs per core
TOK = NP * N             # 2048 tokens per core

DT = BF16


def _legalize_waits(bir: bytes) -> bytes:
    """TRN2 instructions carry a single HW wait slot and this walrus build
    refuses to split multi-wait instructions; hoist extra waits onto NoOps
    inserted just before, on the same engine stream."""
    j = json.loads(bir)
    ctr = 0
    for fn in j["functions"]:
        for blk in fn["blocks"]:
            out = []
            for inst in blk["instructions"]:
                si = inst.get("sync_info")
                if si:
                    waits = si.get("on_wait") or []
                    if len(waits) > 1:
                        for w in waits[:-1]:
                            ctr += 1
                            nop = {
                                "engine": inst["engine"],
                                "ins": [], "outs": [],
                                "name": f"I-waitfix-{ctr}",
                                "opcode": "NoOp",
                                "sync_info": {"on_update": [], "on_wait": [w]},
                            }
                            if "debug" in inst:
                                nop["debug"] = inst["debug"]
                            out.append(nop)
                        si["on_wait"] = waits[-1:]
                out.append(inst)
            blk["instructions"] = out
    return json.dumps(j).encode()


def build(with_bias=True):
    nc = bass.Bass(trn_type="TRN2")
    xt = nc.dram_tensor("xt", [DIM, TOK], BF16, kind="ExternalInput")
    wqk = nc.dram_tensor("wqk", [DIM, 1024], BF16, kind="ExternalInput")
    wv = nc.dram_tensor("wv", [DIM, 512], BF16, kind="ExternalInput")
    wout = nc.dram_tensor("wout", [DIM, 512], BF16, kind="ExternalInput")
    bout = nc.dram_tensor("bout", [1, 512], F32, kind="ExternalInput")
    y = nc.dram_tensor("y", [TOK, DIM], F32, kind="ExternalOutput")
    # den bounce scratch, one per (pair, it); full-tile stores + row-broadcast
    # loads (slice-stores raced the loads)
    dscr = [nc.dram_tensor(f"dscr{t}", [128, 1024], BF16) for t in range(4)]

    with tile.TileContext(nc) as tc, ExitStack() as ctx:
        const = ctx.enter_context(tc.tile_pool(name="const", bufs=1))
        qk_pool = ctx.enter_context(tc.tile_pool(name="qk", bufs=1))
        vv_pool = ctx.enter_context(tc.tile_pool(name="vv", bufs=1))
        et_pool = ctx.enter_context(tc.tile_pool(name="et", bufs=3))
        otn_pool = ctx.enter_context(tc.tile_pool(name="otn", bufs=1))
        den_pool = ctx.enter_context(tc.tile_pool(name="den", bufs=1))
        rb_pool = ctx.enter_context(tc.tile_pool(name="rb", bufs=4))
        y_pool = ctx.enter_context(tc.tile_pool(name="yo", bufs=3))
        ps = ctx.enter_context(tc.tile_pool(name="ps", bufs=1, space="PSUM"))

        # ---- weights / x / bias ----
        wqk_sb = [const.tile([128, 1024], BF16, tag=f"wqk{k}", name=f"wqk{k}")
                  for k in range(4)]
        wv_sb = [const.tile([128, 512], BF16, tag=f"wv{k}", name=f"wv{k}")
                 for k in range(4)]
        wout_sb = [const.tile([128, 512], BF16, tag=f"wout{k}", name=f"wout{k}")
                   for k in range(4)]
        xt_sb = [const.tile([128, TOK], BF16, tag=f"xt{k}", name=f"xt{k}")
                 for k in range(4)]
        bias_sb = const.tile([128, 512], F32, tag="bias", name="bias")
        for k in range(4):
            nc.sync.dma_start(wqk_sb[k][:], wqk.ap()[k * 128:(k + 1) * 128, :])
        for k in range(4):
            nc.sync.dma_start(xt_sb[k][:, 0:N], xt.ap()[k * 128:(k + 1) * 128, 0:N])
        for k in range(4):
            nc.sync.dma_start(wv_sb[k][:], wv.ap()[k * 128:(k + 1) * 128, :])
        for k in range(4):
            nc.sync.dma_start(xt_sb[k][:, N:TOK], xt.ap()[k * 128:(k + 1) * 128, N:TOK])
        for k in range(4):
            nc.sync.dma_start(wout_sb[k][:], wout.ap()[k * 128:(k + 1) * 128, :])
        if with_bias:
            nc.sync.dma_start(bias_sb[:], bout.ap().broadcast_to([128, 512]))

        ones8_f = const.tile([128, 8], F32, tag="ones8f", name="ones8f")
        nc.gpsimd.memset(ones8_f[:], 1.0)
        ones8 = const.tile([128, 8], DT, tag="ones8", name="ones8")
        nc.vector.tensor_copy(ones8[:], ones8_f[:])
        ones_d = const.tile([128, 1024], F32, tag="ones_d", name="ones_d")
        nc.gpsimd.memset(ones_d[:], 1.0)

        # per-pair on-chip tensors
        qkT = {p: [qk_pool.tile([128, N], DT, tag=f"qkT{p}_{c}", name=f"qkT{p}_{c}")
                   for c in range(8)] for p in range(NP)}
        vv = {p: [vv_pool.tile([128, 8 * 65], DT, tag=f"vv{p}_{t}", name=f"vv{p}_{t}")
                  for t in range(8)] for p in range(NP)}
        otn = {p: [otn_pool.tile([128, N], DT, tag=f"otn{p}_{h}", name=f"otn{p}_{h}")
                   for h in range(4)] for p in range(NP)}
        otn_u = {p: [otn_pool.tile([128, N], DT, tag=f"otu{p}_{h}", name=f"otu{p}_{h}")
                     for h in range(4)] for p in range(NP)}
        # den rows live at 32-aligned partitions (engine APs require it):
        # row 32*hp, cols 0:512 head-A den, cols 512:1024 head-B den.
        den = {(p, it): den_pool.tile([128, 1024], F32, tag=f"den{p}_{it}",
                                      name=f"den{p}_{it}")
               for p in range(NP) for it in range(2)}
        rcpf = {(p, it): den_pool.tile([128, 1024], F32, tag=f"rcf{p}_{it}",
                                       name=f"rcf{p}_{it}")
                for p in range(NP) for it in range(2)}
        rcpb = {(p, it): den_pool.tile([128, 1024], BF16, tag=f"rcb{p}_{it}",
                                       name=f"rcb{p}_{it}")
                for p in range(NP) for it in range(2)}
        for t in den.values():
            # unwritten lanes must not be denormal garbage: the gpsimd
            # software divide reads the whole tile
            nc.gpsimd.memset(t[:], 1.0)

        # ---------- filler chains (projections + out-proj), pumped into B ----
        def chain_qk(p, cht, nt):
            """qkT[p][cht][:, nt*512:(nt+1)*512] ch-major projection chain."""
            def emit(on_act=False):
                t0 = p * N
                pa = ps.tile([128, 512], F32, tag="pa", name="pa", bufs=2)
                for kt in range(4):
                    nc.tensor.matmul(
                        pa[:],
                        wqk_sb[kt][:, cht * 128:(cht + 1) * 128],
                        xt_sb[kt][:, t0 + nt * 512:t0 + (nt + 1) * 512],
                        start=(kt == 0), stop=(kt == 3))
                dst = qkT[p][cht][:, nt * 512:(nt + 1) * 512]
                if on_act:
                    nc.scalar.copy(dst, pa[:])
                else:
                    nc.vector.tensor_copy(dst, pa[:])
            return emit

        def chain_v(p, tt):
            """vv[p][tt]: tok-major v, heads packed at stride 65 + ones col."""
            def emit(on_act=False):
                t0 = p * N
                pv = ps.tile([128, 512], F32, tag="pa", name="pa", bufs=2)
                for kt in range(4):
                    nc.tensor.matmul(
                        pv[:],
                        xt_sb[kt][:, t0 + tt * 128:t0 + (tt + 1) * 128],
                        wv_sb[kt][:],
                        start=(kt == 0), stop=(kt == 3))
                dst = vv[p][tt][:].rearrange("p (h c) -> p h c", c=65)[:, :, 0:64]
                src = pv[:].rearrange("p (h c) -> p h c", c=64)
                nc.vector.tensor_copy(dst, src)
                ones_dst = vv[p][tt][:].rearrange("p (h c) -> p h c", c=65)[:, :, 64:65]
                ones_src = ones8[:].rearrange("p (h c) -> p h c", c=1)
                nc.vector.tensor_copy(ones_dst, ones_src)
            return emit

        def chain_c(p, it, tt):
            """out projection for token chunk (p, it, tt) + bias + store."""
            def emit(on_act=False):
                py = ps.tile([128, 512], F32, tag="pa", name="pa", bufs=2)
                for kt in range(4):
                    nc.tensor.matmul(
                        py[:],
                        otn[p][kt][:, it * 512 + tt * 128:it * 512 + (tt + 1) * 128],
                        wout_sb[kt][:],
                        start=(kt == 0), stop=(kt == 3))
                ysb = y_pool.tile([128, 512], F32, tag="ysb", name="ysb")
                if with_bias:
                    nc.vector.tensor_add(ysb[:], py[:], bias_sb[:])
                else:
                    nc.vector.tensor_copy(ysb[:], py[:])
                r0 = p * N + it * 512 + tt * 128
                nc.sync.dma_start(y.ap()[r0:r0 + 128, :], ysb[:])
            return emit

        # ---------- stage B ----------
        def emit_B(p, it, hp, pump):
            """Software-pipelined: sims(jt+1) emitted BEFORE AVs(jt) so the
            in-order PE never idles on exp(jt); psAB is a 2-slot [128,1024]
            ring inside one 4-bank tile (exp reads one slot while sims fill
            the other)."""
            hA, hB = 2 * hp, 2 * hp + 1
            qt, kt_t = qkT[p][hp], qkT[p][4 + hp]
            po_A = ps.tile([128, 512], F32, tag="poA", name="poA", bufs=1)
            po_B = ps.tile([128, 512], F32, tag="poB", name="poB", bufs=1)
            psAB = ps.tile([128, 2048], F32, tag="psAB", name="psAB", bufs=1)
            ets = {}

            def sims(jt):
                h = (jt % 2) * 1024
                nc.tensor.matmul(
                    psAB[:, h:h + 512],
                    kt_t[0:64, jt * 128:(jt + 1) * 128],
                    qt[0:64, it * 512:(it + 1) * 512],
                    start=True, stop=True, tile_position=(0, 0))
                nc.tensor.matmul(
                    psAB[:, h + 512:h + 1024],
                    kt_t[64:128, jt * 128:(jt + 1) * 128],
                    qt[64:128, it * 512:(it + 1) * 512],
                    start=True, stop=True, tile_position=(64, 0))

            def ex(jt):
                h = (jt % 2) * 1024
                et = et_pool.tile([128, 1024], DT, tag="et", name="et")
                nc.scalar.activation(et[:], psAB[:, h:h + 1024], AF.Exp)
                ets[jt] = et

            def avs(jt):
                et = ets.pop(jt)
                nc.tensor.matmul(
                    po_A[0:65, :], vv[p][jt][:, hA * 65:hA * 65 + 65],
                    et[:, 0:512], start=(jt == 0), stop=(jt == 7))
                nc.tensor.matmul(
                    po_B[0:65, :], vv[p][jt][:, hB * 65:hB * 65 + 65],
                    et[:, 512:1024], start=(jt == 0), stop=(jt == 7))

            sims(0)
            ex(0)
            for jt in range(8):
                if jt < 7:
                    sims(jt + 1)
                    ex(jt + 1)
                avs(jt)
                if jt % 2 == 1:
                    pump()
            # denominators + numerator eviction (frees po banks)
            nc.vector.tensor_copy(den[(p, it)][32 * hp:32 * hp + 1, 0:512],
                                  po_A[64:65, :])
            nc.vector.tensor_copy(den[(p, it)][32 * hp:32 * hp + 1, 512:1024],
                                  po_B[64:65, :])
            nc.vector.tensor_copy(otn_u[p][hp][0:64, it * 512:(it + 1) * 512],
                                  po_A[0:64, :])
            nc.vector.tensor_copy(otn_u[p][hp][64:128, it * 512:(it + 1) * 512],
                                  po_B[0:64, :])

        def emit_norm(p, it):
            """recip + DRAM-bounce broadcast + per-hp normalize for (p, it).

            Only the 32-aligned rows of den/rcp tiles hold data; the recip of
            the untouched lanes is garbage that is never read."""
            sc = dscr[p * 2 + it]
            nc.vector.reciprocal(rcpf[(p, it)][:], den[(p, it)][:])
            nc.vector.tensor_copy(rcpb[(p, it)][:], rcpf[(p, it)][:])
            nc.sync.dma_start(sc.ap(), rcpb[(p, it)][:])
            for hp in range(4):
                rb = rb_pool.tile([128, 512], BF16, tag="rb", name="rb")
                nc.sync.dma_start(
                    rb[0:64, :],
                    sc.ap()[32 * hp:32 * hp + 1, 0:512].broadcast_to([64, 512]))
                nc.sync.dma_start(
                    rb[64:128, :],
                    sc.ap()[32 * hp:32 * hp + 1, 512:1024].broadcast_to([64, 512]))
                nc.vector.tensor_mul(otn[p][hp][:, it * 512:(it + 1) * 512],
                                     otn_u[p][hp][:, it * 512:(it + 1) * 512],
                                     rb[:])

        # ---------- schedule ----------
        # A(p) chain order, with the index every B(p, it, hp) requires
        # emitted beforehand (emission order IS dependency order for Tile):
        #   0-10 : v0..v7, k(hp0) both halves, q(hp0) it0  -> B(p,0,0)
        #   11-13: k(hp1), q(hp1) it0                      -> B(p,0,1)  etc.
        #   20+hp: q(hp) it1                               -> B(p,1,hp)
        def a_chains(p):
            out = [(chain_v(p, t), False) for t in range(8)]
            out += [(chain_qk(p, 4, 0), True), (chain_qk(p, 4, 1), True),
                    (chain_qk(p, 0, 0), True)]
            for hp in range(1, 4):
                out += [(chain_qk(p, 4 + hp, 0), False),
                        (chain_qk(p, 4 + hp, 1), False),
                        (chain_qk(p, hp, 0), False)]
            out += [(chain_qk(p, hp, 1), False) for hp in range(4)]
            return out

        Aq = {0: a_chains(0), 1: a_chains(1)}
        Ai = {0: 0, 1: 0}
        Cq = []

        def drainA(p, n, act_assist=False):
            while Ai[p] < n:
                fn, is_qk = Aq[p][Ai[p]]
                fn(on_act=(act_assist and is_qk))
                Ai[p] += 1

        def pump():
            if Ai[0] < len(Aq[0]):
                Aq[0][Ai[0]][0]()
                Ai[0] += 1
            elif Ai[1] < len(Aq[1]):
                Aq[1][Ai[1]][0]()
                Ai[1] += 1
            elif Cq:
                Cq.pop(0)()

        def req(it, hp):
            return (11 + 3 * hp) if it == 0 else (21 + hp)

        def run_B(p, it):
            for hp in range(4):
                drainA(p, req(it, hp), act_assist=(p == 0 and it == 0))
                emit_B(p, it, hp, pump)
            emit_norm(p, it)

        run_B(0, 0)
        run_B(0, 1)
        Cq += [chain_c(0, 0, tt) for tt in range(4)]
        run_B(1, 0)
        Cq += [chain_c(0, 1, tt) for tt in range(4)]
        Cq += [chain_c(1, 0, tt) for tt in range(4)]
        run_B(1, 1)
        while Ai[0] < len(Aq[0]) or Ai[1] < len(Aq[1]) or Cq:
            pump()
        for tt in range(4):
            chain_c(1, 1, tt)()

    _orig = nc.to_json_bytes
    nc.to_json_bytes = lambda: _legalize_waits(_orig())
    return nc


_NC_CACHE = []
_last_in_maps = None


def kernel(**inputs) -> np.ndarray:
    import ml_dtypes
    BF = ml_dtypes.bfloat16

    x = np.ascontiguousarray(np.asarray(inputs["x"], dtype=np.float32))
    W_qkv = np.asarray(inputs["W_qkv"], dtype=np.float32)
    W_out = np.ascontiguousarray(np.asarray(inputs["W_out"], dtype=np.float32))
    b_out = np.ascontiguousarray(np.asarray(inputs["b_out"], dtype=np.float32))
    f = int(np.asarray(inputs["f"]))
    assert f == F and x.shape == (B, NTOT, DIM)

    Wqk = np.ascontiguousarray(np.concatenate(
        [W_qkv[:, :512] * SCALE, W_qkv[:, 512:1024]], axis=1).astype(BF))
    Wv = np.ascontiguousarray(W_qkv[:, 1024:1536].astype(BF))
    Wo = np.ascontiguousarray(W_out.astype(BF))
    bo = np.ascontiguousarray(b_out.reshape(1, 512))

    with_bias = bool(np.any(b_out))
    key = with_bias
    if not _NC_CACHE or _NC_CACHE[0][0] != key:
        _NC_CACHE.clear()
        _NC_CACHE.append((key, build(with_bias)))
    nc = _NC_CACHE[0][1]

    in_maps = []
    for core in range(8):
        pairs = (2 * core, 2 * core + 1)
        xT = np.concatenate(
            [x[p // F, (p % F) * N:(p % F + 1) * N, :].T for p in pairs],
            axis=1).astype(BF)
        in_maps.append({
            "xt": np.ascontiguousarray(xT),
            "wqk": Wqk, "wv": Wv, "wout": Wo, "bout": bo,
        })

    global _last_in_maps
    _last_in_maps = in_maps
    try:
        res = run_bass_kernel_spmd(nc, in_maps, list(range(8)))
    except Exception:
        # transient NRT_EXEC_UNIT_UNRECOVERABLE occasionally hits the first
        # submission after a fresh compile; one retry has always cleared it
        import time
        time.sleep(10)
        res = run_bass_kernel_spmd(nc, in_maps, list(range(8)))

    out = np.zeros((B, NTOT, DIM), dtype=np.float32)
    for core in range(8):
        yc = res.results[core]["y"]
        for pi, p in enumerate((2 * core, 2 * core + 1)):
            out[p // F, (p % F) * N:(p % F + 1) * N, :] = yc[pi * N:(pi + 1) * N]
    return out
